# revision 1
# baseline (speedup 1.0000x reference)
"""Bass/Trainium2 kernel for the DGPE relaxation RHS on a 192^3 periodic lattice.

The nn_id* inputs are the deterministic 6-neighbor roll indices of the
lattice, so the gathers are implemented as stencil shifts.  The lattice is
sharded along axis 0 across 8 NeuronCores (24 planes + 2 halo planes each,
sliced host-side).  Within a core, partition = (k-block, j-block) = 8 x 16,
each partition holding a (24 x 12 x 24) sub-brick stored with j/k halo
strips so every neighbor access is a plain access-pattern offset.
"""

import numpy as np

L = 192
N = L ** 3
NCORES = 8
CH = L // NCORES            # 24 planes (axis 0) per core
KH, JB = 8, 16              # partition grid: p = kh*JB + jb
JW = L // JB                # 12 j's per partition
KW = L // KH                # 24 k's per partition
IH = CH + 2                 # 26 planes incl. axis-0 halo
FJ = JW + 2                 # 14 incl. j halo strips
FK = KW + 2                 # 26 incl. k halo strips
PLF = FJ * FK               # padded plane free size (364)
FIN = IH * PLF
PF = JW * KW                # compact plane free size (288)
FOUT = CH * PF
T = 8                       # planes per compute tile
NT = CH // T
TF = T * PF

_STATE = {}


# ---------------------------------------------------------------- host side

def _shard_halo(v3pad):
    """(194,194,194) wrap-padded -> (8, 128, FIN) per-core images."""
    s0, s1, s2 = v3pad.strides
    v = np.lib.stride_tricks.as_strided(
        v3pad,
        shape=(NCORES, KH, JB, IH, FJ, FK),
        strides=(CH * s0, KW * s2, JW * s1, s0, s1, s2),
    )
    return np.ascontiguousarray(v).reshape(NCORES, 128, FIN)


def _shard_compact(v3):
    """(192,192,192) -> (8, 128, CH, PF) per-core compact images."""
    s0, s1, s2 = v3.strides
    v = np.lib.stride_tricks.as_strided(
        v3,
        shape=(NCORES, KH, JB, CH, JW, KW),
        strides=(CH * s0, KW * s2, JW * s1, s0, s1, s2),
    )
    return np.ascontiguousarray(v).reshape(NCORES, 128, CH, PF)


def _unshard_compact(per_core):
    """(8, 128, CH*PF) -> (192,192,192)."""
    out3 = np.empty((L, L, L), np.float32)
    s0, s1, s2 = out3.strides
    w = np.lib.stride_tricks.as_strided(
        out3,
        shape=(NCORES, KH, JB, CH, JW, KW),
        strides=(CH * s0, KW * s2, JW * s1, s0, s1, s2),
    )
    w[:] = per_core.reshape(NCORES, KH, JB, CH, JW, KW)
    return out3


def _is_const(a):
    a = np.asarray(a)
    return bool(a.size) and bool(np.all(a == a.flat[0]))


def _rolls_ok(nn_idx_1, nn_idx_2, nn_idy_1, nn_idy_2, nn_idz_1, nn_idz_2):
    """Spot-check that the index arrays are the periodic roll stencil."""
    rng = np.random.default_rng(12345)
    f = rng.integers(0, N, size=4096)
    i, r = np.divmod(f, L * L)
    j, k = np.divmod(r, L)

    def flat(ii, jj, kk):
        return (ii % L) * L * L + (jj % L) * L + (kk % L)

    checks = [
        (nn_idx_1, flat(i - 1, j, k)), (nn_idx_2, flat(i + 1, j, k)),
        (nn_idy_1, flat(i, j - 1, k)), (nn_idy_2, flat(i, j + 1, k)),
        (nn_idz_1, flat(i, j, k - 1)), (nn_idz_2, flat(i, j, k + 1)),
    ]
    for arr, want in checks:
        if not np.array_equal(np.asarray(arr)[f], want):
            return False
    return True


def _numpy_fallback(y, J, anisotropy, gamma, h_dis_x, h_dis_y, beta,
                    e_disorder, idx):
    """Exact reference math in numpy (used only if structure checks fail)."""
    x, p = y[:N], y[N:]

    def stencil(v):
        return J * (v[idx[0]] + v[idx[1]] + v[idx[2]] + v[idx[3]]
                    + anisotropy * (v[idx[4]] + v[idx[5]]))

    xL = stencil(x)
    yL = stencil(p)
    r2 = x * x + p * p
    cross = xL * p - yL * x
    dx = gamma * p * cross + e_disorder * p - yL + h_dis_y + beta * r2 * p
    dp = -gamma * x * cross - e_disorder * x + xL - h_dis_x - beta * r2 * x
    return np.concatenate([dx, dp]).astype(np.float32)


# -------------------------------------------------------------- device side

def _build_nc():
    from concourse import bacc
    import concourse.mybir as mybir
    from concourse.mybir import AluOpType as Op
    from concourse.tile import TileContext, add_dep_helper

    ActF = mybir.ActivationFunctionType
    f32 = mybir.dt.float32

    nc = bacc.Bacc("TRN2", target_bir_lowering=False, debug=False,
                   enable_asserts=False, num_devices=NCORES)
    x_in = nc.dram_tensor("x_in", [128, FIN], f32, kind="ExternalInput").ap()
    p_in = nc.dram_tensor("p_in", [128, FIN], f32, kind="ExternalInput").ap()
    # packed per-tile coefficients: [e_disorder | h_dis_x | h_dis_y]
    cf_in = nc.dram_tensor("cf_in", [128, NT, 3, TF], f32, kind="ExternalInput").ap()
    cst_in = nc.dram_tensor("cst_in", [128, 8], f32, kind="ExternalInput").ap()
    dx_out = nc.dram_tensor("dx_out", [128, FOUT], f32, kind="ExternalOutput").ap()
    dp_out = nc.dram_tensor("dp_out", [128, FOUT], f32, kind="ExternalOutput").ap()

    with TileContext(nc) as tc:
        with (
            tc.tile_pool(name="persist", bufs=1) as pers,
            tc.tile_pool(name="state", bufs=2) as sp,
            tc.tile_pool(name="coef", bufs=1) as cp,
            tc.tile_pool(name="outs", bufs=2) as op_,
            tc.tile_pool(name="ubuf", bufs=2) as up,
            tc.tile_pool(name="tmp", bufs=1) as tp,
        ):
            cst = pers.tile([128, 8], f32, name="cst")
            ANIS = cst[:, 0:1]   # anisotropy
            GJ = cst[:, 1:2]     # gamma * J
            BET = cst[:, 2:3]    # beta
            JC = cst[:, 3:4]     # J
            NJC = cst[:, 4:5]    # -J

            for t in range(NT):
                i0 = t * T
                f0 = i0 * PF

                xt = sp.tile([128, (T + 2) * PLF], f32, tag="xt", name=f"xt{t}")
                if t == 0:
                    Hx = (T + 2) * PLF // 2
                    nc.sync.dma_start(xt[:, :Hx], x_in[:, :Hx])
                    nc.sync.dma_start(xt[:, Hx:], x_in[:, Hx:(T + 2) * PLF])
                else:
                    nc.sync.dma_start(xt[:], x_in[:, i0 * PLF:(i0 + T + 2) * PLF])
                pt = sp.tile([128, (T + 2) * PLF], f32, tag="pt", name=f"pt{t}")
                nc.sync.dma_start(pt[:], p_in[:, i0 * PLF:(i0 + T + 2) * PLF])
                if t == 0:
                    nc.sync.dma_start(cst[:], cst_in)
                ct = cp.tile([128, 3, TF], f32, tag="ct", name=f"ct{t}")
                nc.sync.dma_start(ct[:], cf_in[:, t])
                ed, hx, hy = ct[:, 0, :], ct[:, 1, :], ct[:, 2, :]

                def sl(img, di, dj, dk):
                    v = img[:].rearrange("q (i j k) -> q i j k",
                                         i=T + 2, j=FJ, k=FK)
                    return v[:, 1 + di: 1 + T + di,
                             1 + dj: 1 + JW + dj, 1 + dk: 1 + KW + dk]

                xc = sl(xt, 0, 0, 0)
                pc = sl(pt, 0, 0, 0)

                def v4(tile):
                    return tile[:].rearrange("q (i j k) -> q i j k",
                                             i=T, j=JW, k=KW)

                S1 = tp.tile([128, TF], f32, tag="S1", name=f"S1_{t}")
                S2 = tp.tile([128, TF], f32, tag="S2", name=f"S2_{t}")
                S3 = tp.tile([128, TF], f32, tag="S3", name=f"S3_{t}")
                S4 = tp.tile([128, TF], f32, tag="S4", name=f"S4_{t}")
                S5 = tp.tile([128, TF], f32, tag="S5", name=f"S5_{t}")

                # ---- x stencil: vx = (i-sum) + (j-sum) + anis*(k-sum)
                nc.vector.tensor_add(v4(S1), sl(xt, -1, 0, 0), sl(xt, 1, 0, 0))
                nc.vector.tensor_add(v4(S2), sl(xt, 0, -1, 0), sl(xt, 0, 1, 0))
                nc.vector.tensor_add(v4(S5), sl(xt, 0, 0, -1), sl(xt, 0, 0, 1))

                # ---- r2 = x^2 + p^2: squares on ACT (own SBUF ports, runs
                # alongside the DVE stencil work).  GpSimd compute and DMA
                # accumulates are avoided: the former locks the shared SBUF
                # port (~3x DVE slowdown), the latter proved fragile.
                nc.scalar.activation(v4(S3), xc, ActF.Square)
                nc.scalar.activation(v4(S4), pc, ActF.Square)
                nc.vector.tensor_add(S3[:], S3[:], S4[:])            # r2

                nc.vector.scalar_tensor_tensor(S5[:], S5[:], ANIS, S2[:], Op.mult, Op.add)
                nc.vector.tensor_add(S1[:], S5[:], S1[:])            # vx

                # ---- p stencil
                nc.vector.tensor_add(v4(S2), sl(pt, -1, 0, 0), sl(pt, 1, 0, 0))
                nc.vector.tensor_add(v4(S5), sl(pt, 0, -1, 0), sl(pt, 0, 1, 0))
                nc.vector.tensor_add(v4(S4), sl(pt, 0, 0, -1), sl(pt, 0, 0, 1))
                nc.vector.scalar_tensor_tensor(S4[:], S4[:], ANIS, S5[:], Op.mult, Op.add)
                nc.vector.tensor_add(S2[:], S4[:], S2[:])            # vy

                # ---- cross_raw = vx*p - vy*x
                nc.vector.tensor_mul(v4(S4), v4(S2), xc)             # w2 = vy*x
                nc.vector.tensor_mul(v4(S5), v4(S1), pc)             # w1 = vx*p
                nc.vector.tensor_sub(S4[:], S5[:], S4[:])            # cross_raw

                # ---- s2 = (gamma*J)*cross_raw + e_dis + beta*r2
                nc.vector.scalar_tensor_tensor(S4[:], S4[:], GJ, ed, Op.mult, Op.add)
                nc.vector.scalar_tensor_tensor(S4[:], S3[:], BET, S4[:], Op.mult, Op.add)

                # ---- dx = p*s2 + (h_y - J*vy)
                dxo = op_.tile([128, TF], f32, tag="dxo", name=f"dxo{t}")
                nc.vector.tensor_mul(v4(dxo), pc, v4(S4))            # t1
                if t < NT - 1:
                    # final add rides the store: plain store of t1, then a
                    # CCE accumulate of u1 into the same DRAM range (split
                    # to respect the 2048-elem CCE descriptor limit)
                    u1t = up.tile([128, TF], f32, tag="u1t", name=f"u1t{t}")
                    nc.vector.scalar_tensor_tensor(u1t[:], S2[:], NJC, hy, Op.mult, Op.add)  # u1
                    st1 = nc.sync.dma_start(dx_out[:, f0:f0 + TF], dxo[:])
                    Hh = TF // 2
                    for lo, hi in ((0, Hh), (Hh, TF)):
                        a = nc.gpsimd.dma_start(dx_out[:, f0 + lo:f0 + hi],
                                                u1t[:, lo:hi], accum_op=Op.add)
                        add_dep_helper(a.ins, st1.ins, reason="dram RMW after t1 store")
                else:
                    nc.vector.scalar_tensor_tensor(S5[:], S2[:], NJC, hy, Op.mult, Op.add)  # u1
                    nc.vector.tensor_add(dxo[:], dxo[:], S5[:])
                    nc.sync.dma_start(dx_out[:, f0:f0 + TF], dxo[:])

                # ---- dp = (J*vx - h_x) - x*s2
                dpo = op_.tile([128, TF], f32, tag="dpo", name=f"dpo{t}")
                if t == NT - 1:
                    nc.vector.scalar_tensor_tensor(S1[:], S1[:], JC, hx, Op.mult, Op.subtract)  # u2
                    nc.vector.tensor_mul(v4(dpo), xc, v4(S4))        # t2
                    Hh = TF // 2
                    nc.vector.tensor_sub(dpo[:, :Hh], S1[:, :Hh], dpo[:, :Hh])
                    nc.sync.dma_start(dp_out[:, f0:f0 + Hh], dpo[:, :Hh])
                    nc.vector.tensor_sub(dpo[:, Hh:], S1[:, Hh:], dpo[:, Hh:])
                    nc.sync.dma_start(dp_out[:, f0 + Hh:f0 + TF], dpo[:, Hh:])
                else:
                    # dp = u2 + (-x*s2): store u2, negate s2 in place (2x
                    # single-src op, after t1 consumed it), then accumulate
                    u2t = up.tile([128, TF], f32, tag="u2t", name=f"u2t{t}")
                    nc.vector.scalar_tensor_tensor(u2t[:], S1[:], JC, hx, Op.mult, Op.subtract)  # u2
                    st2 = nc.sync.dma_start(dp_out[:, f0:f0 + TF], u2t[:])
                    nc.vector.tensor_scalar_mul(S4[:], S4[:], -1.0)  # -s2
                    nc.vector.tensor_mul(v4(dpo), xc, v4(S4))        # -t2
                    Hh = TF // 2
                    for lo, hi in ((0, Hh), (Hh, TF)):
                        a = nc.gpsimd.dma_start(dp_out[:, f0 + lo:f0 + hi],
                                                dpo[:, lo:hi], accum_op=Op.add)
                        add_dep_helper(a.ins, st2.ins, reason="dram RMW after u2 store")

    nc.compile()
    return nc


def _get_nc():
    if "nc" not in _STATE:
        _STATE["nc"] = _build_nc()
    return _STATE["nc"]


def _run(in_maps, trace=False, trace_cores=None):
    from concourse.bass_utils import run_bass_kernel_spmd
    if trace:
        # the agent image's antenv lacks axon_hooks; wire the NTFF hook
        import sys as _sys
        import types as _types
        if "antenv.axon_hooks" not in _sys.modules:
            try:
                import trn_agent_boot.trn_boot as _tb
                _hook = _tb._ntff_profile_via_ctypes('/opt/axon/libaxon_pjrt.so')
                _mod = _types.ModuleType("antenv.axon_hooks")
                _mod.get_axon_ntff_profile_hook = lambda: _hook
                _sys.modules["antenv.axon_hooks"] = _mod
            except Exception:
                pass
    return run_bass_kernel_spmd(
        _get_nc(), in_maps, core_ids=list(range(NCORES)),
        trace=trace, trace_cores=trace_cores,
    )


def prepare_in_maps(y, anis_v, gamma_v, beta_v, j_v, h_dis_x, h_dis_y,
                    e_disorder):
    """Host-side sharding: build the 8 per-core input maps."""
    x3 = np.ascontiguousarray(y[:N], np.float32).reshape(L, L, L)
    p3 = np.ascontiguousarray(y[N:], np.float32).reshape(L, L, L)
    xs = _shard_halo(np.pad(x3, 1, mode="wrap"))
    ps = _shard_halo(np.pad(p3, 1, mode="wrap"))
    eds = _shard_compact(np.ascontiguousarray(e_disorder, np.float32).reshape(L, L, L))
    hxs = _shard_compact(np.ascontiguousarray(h_dis_x, np.float32).reshape(L, L, L))
    hys = _shard_compact(np.ascontiguousarray(h_dis_y, np.float32).reshape(L, L, L))
    # pack per-tile coefficient blocks: [NT, 3, TF]
    cf = np.stack([eds.reshape(NCORES, 128, NT, TF),
                   hxs.reshape(NCORES, 128, NT, TF),
                   hys.reshape(NCORES, 128, NT, TF)], axis=3)
    cf = np.ascontiguousarray(cf)          # (8, 128, NT, 3, TF)
    cst = np.zeros((128, 8), np.float32)
    cst[:, 0] = anis_v
    cst[:, 1] = gamma_v * j_v
    cst[:, 2] = beta_v
    cst[:, 3] = j_v
    cst[:, 4] = -j_v
    return [
        {"x_in": xs[c], "p_in": ps[c], "cf_in": cf[c], "cst_in": cst}
        for c in range(NCORES)
    ]


def assemble_output(results):
    """Per-core device outputs -> full (2N,) float32 array."""
    dxs = np.stack([results[c]["dx_out"] for c in range(NCORES)])
    dps = np.stack([results[c]["dp_out"] for c in range(NCORES)])
    dx3 = _unshard_compact(dxs)
    dp3 = _unshard_compact(dps)
    return np.concatenate([dx3.reshape(-1), dp3.reshape(-1)])


def kernel(t, y, J, anisotropy, gamma, h_dis_x, h_dis_y, beta, e_disorder,
           nn_idx_1, nn_idx_2, nn_idy_1, nn_idy_2, nn_idz_1, nn_idz_2):
    y = np.asarray(y, np.float32)
    J = np.asarray(J, np.float32)
    anisotropy = np.asarray(anisotropy, np.float32)
    gamma = np.asarray(gamma, np.float32)
    beta = np.asarray(beta, np.float32)
    h_dis_x = np.asarray(h_dis_x, np.float32)
    h_dis_y = np.asarray(h_dis_y, np.float32)
    e_disorder = np.asarray(e_disorder, np.float32)

    ok = (y.shape == (2 * N,)
          and _is_const(J) and _is_const(anisotropy)
          and _is_const(gamma) and _is_const(beta)
          and _rolls_ok(nn_idx_1, nn_idx_2, nn_idy_1, nn_idy_2,
                        nn_idz_1, nn_idz_2))
    if not ok:
        idx = [np.asarray(a) for a in (nn_idx_1, nn_idx_2, nn_idy_1,
                                       nn_idy_2, nn_idz_1, nn_idz_2)]
        return _numpy_fallback(y, J, anisotropy, gamma, h_dis_x, h_dis_y,
                               beta, e_disorder, idx)

    in_maps = prepare_in_maps(
        y, float(anisotropy.flat[0]), float(gamma.flat[0]),
        float(beta.flat[0]), float(J.flat[0]), h_dis_x, h_dis_y, e_disorder)
    res = _run(in_maps, trace=False)
    return assemble_output(res.results)



# revision 4
# speedup vs baseline: 1.4318x; 1.4318x over previous
"""Bass/Trainium2 kernel for the DGPE relaxation RHS on a 192^3 periodic lattice.

The nn_id* inputs are the deterministic 6-neighbor roll indices of the
lattice, so the gathers are implemented as stencil shifts.  The lattice is
sharded along axis 0 across 8 NeuronCores (24 planes + 2 halo planes each,
sliced host-side).  Within a core, partition = (k-block, j-block) = 8 x 16,
each partition holding a (24 x 12 x 24) sub-brick stored with j/k halo
strips so every neighbor access is a plain access-pattern offset.

All device tensors are fp16: the DVE is SBUF-bandwidth bound, and 2-byte
packed operands double (or quadruple) its throughput while also halving
HBM traffic.  The loose rel-err budget (2e-2) dwarfs fp16 rounding.
"""

import numpy as np

L = 192
N = L ** 3
NCORES = 8
CH = L // NCORES            # 24 planes (axis 0) per core
KH, JB = 8, 16              # partition grid: p = kh*JB + jb
JW = L // JB                # 12 j's per partition
KW = L // KH                # 24 k's per partition
IH = CH + 2                 # 26 planes incl. axis-0 halo
FJ = JW + 2                 # 14 incl. j halo strips
FK = KW + 2                 # 26 incl. k halo strips
PLF = FJ * FK               # padded plane free size (364)
FIN = IH * PLF
PF = JW * KW                # compact plane free size (288)
FOUT = CH * PF
T = 8                       # planes per compute tile
NT = CH // T
TF = T * PF

_STATE = {}


# ---------------------------------------------------------------- host side

def _shard_halo(v3pad):
    """(194,194,194) wrap-padded -> (8, 128, FIN) per-core images."""
    s0, s1, s2 = v3pad.strides
    v = np.lib.stride_tricks.as_strided(
        v3pad,
        shape=(NCORES, KH, JB, IH, FJ, FK),
        strides=(CH * s0, KW * s2, JW * s1, s0, s1, s2),
    )
    return np.ascontiguousarray(v).reshape(NCORES, 128, FIN)


def _shard_compact(v3):
    """(192,192,192) -> (8, 128, CH, PF) per-core compact images."""
    s0, s1, s2 = v3.strides
    v = np.lib.stride_tricks.as_strided(
        v3,
        shape=(NCORES, KH, JB, CH, JW, KW),
        strides=(CH * s0, KW * s2, JW * s1, s0, s1, s2),
    )
    return np.ascontiguousarray(v).reshape(NCORES, 128, CH, PF)


def _unshard_compact(per_core):
    """(8, 128, CH*PF) -> (192,192,192) float32."""
    out3 = np.empty((L, L, L), np.float32)
    s0, s1, s2 = out3.strides
    w = np.lib.stride_tricks.as_strided(
        out3,
        shape=(NCORES, KH, JB, CH, JW, KW),
        strides=(CH * s0, KW * s2, JW * s1, s0, s1, s2),
    )
    w[:] = per_core.reshape(NCORES, KH, JB, CH, JW, KW)
    return out3


def _is_const(a):
    a = np.asarray(a)
    return bool(a.size) and bool(np.all(a == a.flat[0]))


def _rolls_ok(nn_idx_1, nn_idx_2, nn_idy_1, nn_idy_2, nn_idz_1, nn_idz_2):
    """Spot-check that the index arrays are the periodic roll stencil."""
    rng = np.random.default_rng(12345)
    f = rng.integers(0, N, size=4096)
    i, r = np.divmod(f, L * L)
    j, k = np.divmod(r, L)

    def flat(ii, jj, kk):
        return (ii % L) * L * L + (jj % L) * L + (kk % L)

    checks = [
        (nn_idx_1, flat(i - 1, j, k)), (nn_idx_2, flat(i + 1, j, k)),
        (nn_idy_1, flat(i, j - 1, k)), (nn_idy_2, flat(i, j + 1, k)),
        (nn_idz_1, flat(i, j, k - 1)), (nn_idz_2, flat(i, j, k + 1)),
    ]
    for arr, want in checks:
        if not np.array_equal(np.asarray(arr)[f], want):
            return False
    return True


def _numpy_fallback(y, J, anisotropy, gamma, h_dis_x, h_dis_y, beta,
                    e_disorder, idx):
    """Exact reference math in numpy (used only if structure checks fail)."""
    x, p = y[:N], y[N:]

    def stencil(v):
        return J * (v[idx[0]] + v[idx[1]] + v[idx[2]] + v[idx[3]]
                    + anisotropy * (v[idx[4]] + v[idx[5]]))

    xL = stencil(x)
    yL = stencil(p)
    r2 = x * x + p * p
    cross = xL * p - yL * x
    dx = gamma * p * cross + e_disorder * p - yL + h_dis_y + beta * r2 * p
    dp = -gamma * x * cross - e_disorder * x + xL - h_dis_x - beta * r2 * x
    return np.concatenate([dx, dp]).astype(np.float32)


# -------------------------------------------------------------- device side

def _build_nc(anis, gJ, beta, J):
    from concourse import bacc
    import concourse.mybir as mybir
    from concourse.mybir import AluOpType as Op
    from concourse.tile import TileContext

    ActF = mybir.ActivationFunctionType
    f16 = mybir.dt.float16

    nc = bacc.Bacc("TRN2", target_bir_lowering=False, debug=False,
                   enable_asserts=False, num_devices=NCORES)
    x_in = nc.dram_tensor("x_in", [128, FIN], f16, kind="ExternalInput").ap()
    p_in = nc.dram_tensor("p_in", [128, FIN], f16, kind="ExternalInput").ap()
    # packed per-tile coefficients: [e_disorder | h_dis_x | h_dis_y]
    cf_in = nc.dram_tensor("cf_in", [128, NT, 3, TF], f16, kind="ExternalInput").ap()
    dx_out = nc.dram_tensor("dx_out", [128, FOUT], f16, kind="ExternalOutput").ap()
    dp_out = nc.dram_tensor("dp_out", [128, FOUT], f16, kind="ExternalOutput").ap()

    with TileContext(nc) as tc:
        with (
            tc.tile_pool(name="state", bufs=2) as sp,
            tc.tile_pool(name="coef", bufs=2) as cp,
            tc.tile_pool(name="outs", bufs=2) as op_,
            tc.tile_pool(name="tmp", bufs=1) as tp,
        ):
            for t in range(NT):
                f0 = t * T * PF
                i0 = t * T

                xt = sp.tile([128, (T + 2) * PLF], f16, tag="xt", name=f"xt{t}")
                if t == 0:
                    Hx = (T + 2) * PLF // 2
                    nc.sync.dma_start(xt[:, :Hx], x_in[:, :Hx])
                    nc.sync.dma_start(xt[:, Hx:], x_in[:, Hx:(T + 2) * PLF])
                else:
                    nc.sync.dma_start(xt[:], x_in[:, i0 * PLF:(i0 + T + 2) * PLF])
                pt = sp.tile([128, (T + 2) * PLF], f16, tag="pt", name=f"pt{t}")
                nc.sync.dma_start(pt[:], p_in[:, i0 * PLF:(i0 + T + 2) * PLF])
                ct = cp.tile([128, 3, TF], f16, tag="ct", name=f"ct{t}")
                nc.sync.dma_start(ct[:], cf_in[:, t])
                ed, hx, hy = ct[:, 0, :], ct[:, 1, :], ct[:, 2, :]

                def sl(img, di, dj, dk):
                    v = img[:].rearrange("q (i j k) -> q i j k",
                                         i=T + 2, j=FJ, k=FK)
                    return v[:, 1 + di: 1 + T + di,
                             1 + dj: 1 + JW + dj, 1 + dk: 1 + KW + dk]

                xc = sl(xt, 0, 0, 0)
                pc = sl(pt, 0, 0, 0)

                def v4(tile):
                    return tile[:].rearrange("q (i j k) -> q i j k",
                                             i=T, j=JW, k=KW)

                S1 = tp.tile([128, TF], f16, tag="S1", name=f"S1_{t}")
                S2 = tp.tile([128, TF], f16, tag="S2", name=f"S2_{t}")
                S3 = tp.tile([128, TF], f16, tag="S3", name=f"S3_{t}")
                S4 = tp.tile([128, TF], f16, tag="S4", name=f"S4_{t}")
                S5 = tp.tile([128, TF], f16, tag="S5", name=f"S5_{t}")

                # ---- x stencil: sx = (i-sum) + (j-sum) + anis*(k-sum)
                nc.vector.tensor_add(v4(S1), sl(xt, -1, 0, 0), sl(xt, 1, 0, 0))
                nc.vector.tensor_add(v4(S2), sl(xt, 0, -1, 0), sl(xt, 0, 1, 0))
                nc.vector.tensor_add(v4(S5), sl(xt, 0, 0, -1), sl(xt, 0, 0, 1))

                # ---- r2 = x^2 + p^2: squares on ACT (own SBUF ports, runs
                # alongside the DVE stencil work)
                nc.scalar.activation(v4(S3), xc, ActF.Square)
                nc.scalar.activation(v4(S4), pc, ActF.Square)

                nc.vector.scalar_tensor_tensor(S5[:], S5[:], anis, S2[:], Op.mult, Op.add)
                nc.vector.tensor_add(S1[:], S5[:], S1[:])            # sx
                nc.vector.tensor_add(S3[:], S3[:], S4[:])            # r2

                # ---- p stencil
                nc.vector.tensor_add(v4(S2), sl(pt, -1, 0, 0), sl(pt, 1, 0, 0))
                nc.vector.tensor_add(v4(S5), sl(pt, 0, -1, 0), sl(pt, 0, 1, 0))
                nc.vector.tensor_add(v4(S4), sl(pt, 0, 0, -1), sl(pt, 0, 0, 1))
                nc.vector.scalar_tensor_tensor(S4[:], S4[:], anis, S5[:], Op.mult, Op.add)
                nc.vector.tensor_add(S2[:], S4[:], S2[:])            # sp

                # ---- cross_raw = sx*p - sp*x
                nc.vector.tensor_mul(v4(S4), v4(S2), xc)             # w2 = sp*x
                nc.vector.tensor_mul(v4(S5), v4(S1), pc)             # w1 = sx*p
                nc.vector.tensor_sub(S4[:], S5[:], S4[:])            # cross_raw

                # ---- s2 = (gamma*J)*cross_raw + e_dis + beta*r2
                nc.vector.scalar_tensor_tensor(S4[:], S4[:], gJ, ed, Op.mult, Op.add)
                nc.vector.scalar_tensor_tensor(S4[:], S3[:], beta, S4[:], Op.mult, Op.add)

                # ---- dx = p*s2 + (h_y - J*sp)
                dxo = op_.tile([128, TF], f16, tag="dxo", name=f"dxo{t}")
                nc.vector.tensor_mul(v4(dxo), pc, v4(S4))            # t1
                nc.vector.scalar_tensor_tensor(S5[:], S2[:], -J, hy, Op.mult, Op.add)  # u1
                nc.vector.tensor_add(dxo[:], dxo[:], S5[:])
                nc.sync.dma_start(dx_out[:, f0:f0 + TF], dxo[:])

                # ---- dp = (J*sx - h_x) - x*s2
                dpo = op_.tile([128, TF], f16, tag="dpo", name=f"dpo{t}")
                nc.vector.scalar_tensor_tensor(S1[:], S1[:], J, hx, Op.mult, Op.subtract)  # u2
                nc.vector.tensor_mul(v4(dpo), xc, v4(S4))            # t2 = x*s2
                nc.vector.tensor_sub(dpo[:], S1[:], dpo[:])
                nc.sync.dma_start(dp_out[:, f0:f0 + TF], dpo[:])

    nc.compile()
    return nc


def _get_nc():
    key = ("nc",) + tuple(_STATE.get("consts", (0.45, 0.05, 0.01, 1.0)))
    if key not in _STATE:
        anis, gamma, beta, J = _STATE.get("consts", (0.45, 0.05, 0.01, 1.0))
        _STATE[key] = _build_nc(float(anis), float(gamma * J), float(beta),
                                float(J))
    return _STATE[key]


def _run(in_maps, trace=False, trace_cores=None):
    from concourse.bass_utils import run_bass_kernel_spmd
    if trace:
        # the agent image's antenv lacks axon_hooks; wire the NTFF hook
        import sys as _sys
        import types as _types
        if "antenv.axon_hooks" not in _sys.modules:
            try:
                import trn_agent_boot.trn_boot as _tb
                _hook = _tb._ntff_profile_via_ctypes('/opt/axon/libaxon_pjrt.so')
                _mod = _types.ModuleType("antenv.axon_hooks")
                _mod.get_axon_ntff_profile_hook = lambda: _hook
                _sys.modules["antenv.axon_hooks"] = _mod
            except Exception:
                pass
    return run_bass_kernel_spmd(
        _get_nc(), in_maps, core_ids=list(range(NCORES)),
        trace=trace, trace_cores=trace_cores,
    )


def prepare_in_maps(y, anis_v, gamma_v, beta_v, j_v, h_dis_x, h_dis_y,
                    e_disorder):
    """Host-side sharding: build the 8 per-core input maps."""
    _STATE["consts"] = (float(anis_v), float(gamma_v), float(beta_v),
                        float(j_v))
    x3 = np.ascontiguousarray(y[:N], np.float32).reshape(L, L, L)
    p3 = np.ascontiguousarray(y[N:], np.float32).reshape(L, L, L)
    xs = _shard_halo(np.pad(x3, 1, mode="wrap")).astype(np.float16)
    ps = _shard_halo(np.pad(p3, 1, mode="wrap")).astype(np.float16)
    eds = _shard_compact(np.ascontiguousarray(e_disorder, np.float32).reshape(L, L, L))
    hxs = _shard_compact(np.ascontiguousarray(h_dis_x, np.float32).reshape(L, L, L))
    hys = _shard_compact(np.ascontiguousarray(h_dis_y, np.float32).reshape(L, L, L))
    # pack per-tile coefficient blocks: [NT, 3, TF]
    cf = np.stack([eds.reshape(NCORES, 128, NT, TF),
                   hxs.reshape(NCORES, 128, NT, TF),
                   hys.reshape(NCORES, 128, NT, TF)], axis=3)
    cf = np.ascontiguousarray(cf).astype(np.float16)   # (8, 128, NT, 3, TF)
    return [
        {"x_in": xs[c], "p_in": ps[c], "cf_in": cf[c]}
        for c in range(NCORES)
    ]


def assemble_output(results):
    """Per-core device outputs -> full (2N,) float32 array."""
    dxs = np.stack([results[c]["dx_out"] for c in range(NCORES)]).astype(np.float32)
    dps = np.stack([results[c]["dp_out"] for c in range(NCORES)]).astype(np.float32)
    dx3 = _unshard_compact(dxs)
    dp3 = _unshard_compact(dps)
    return np.concatenate([dx3.reshape(-1), dp3.reshape(-1)])


def kernel(t, y, J, anisotropy, gamma, h_dis_x, h_dis_y, beta, e_disorder,
           nn_idx_1, nn_idx_2, nn_idy_1, nn_idy_2, nn_idz_1, nn_idz_2):
    y = np.asarray(y, np.float32)
    J = np.asarray(J, np.float32)
    anisotropy = np.asarray(anisotropy, np.float32)
    gamma = np.asarray(gamma, np.float32)
    beta = np.asarray(beta, np.float32)
    h_dis_x = np.asarray(h_dis_x, np.float32)
    h_dis_y = np.asarray(h_dis_y, np.float32)
    e_disorder = np.asarray(e_disorder, np.float32)

    ok = (y.shape == (2 * N,)
          and _is_const(J) and _is_const(anisotropy)
          and _is_const(gamma) and _is_const(beta)
          and _rolls_ok(nn_idx_1, nn_idx_2, nn_idy_1, nn_idy_2,
                        nn_idz_1, nn_idz_2))
    if not ok:
        idx = [np.asarray(a) for a in (nn_idx_1, nn_idx_2, nn_idy_1,
                                       nn_idy_2, nn_idz_1, nn_idz_2)]
        return _numpy_fallback(y, J, anisotropy, gamma, h_dis_x, h_dis_y,
                               beta, e_disorder, idx)

    in_maps = prepare_in_maps(
        y, float(anisotropy.flat[0]), float(gamma.flat[0]),
        float(beta.flat[0]), float(J.flat[0]), h_dis_x, h_dis_y, e_disorder)
    res = _run(in_maps, trace=False)
    return assemble_output(res.results)


# revision 8
# speedup vs baseline: 1.6217x; 1.1326x over previous
"""Bass/Trainium2 kernel for the DGPE relaxation RHS on a 192^3 periodic lattice.

The nn_id* inputs are the deterministic 6-neighbor roll indices of the
lattice, so the gathers are implemented as stencil shifts.  The lattice is
sharded along axis 0 across 8 NeuronCores (24 planes + 2 halo planes each,
sliced host-side).  Within a core, partition = (k-block, j-block) = 8 x 16,
each partition holding a (24 x 12 x 24) sub-brick stored with j/k halo
strips so every neighbor access is a plain access-pattern offset.

All device tensors are fp16: the DVE is SBUF-bandwidth bound, and 2-byte
packed operands double its throughput while also halving HBM traffic.  The
loose rel-err budget (2e-2) dwarfs fp16 rounding.  Scalar scalings (anis,
gamma*J, beta) ride the ACT engine / the ACT Square scale because the DVE
STT fused ops don't get the 2-byte fast path while plain tensor_tensor
ops do.
"""

import numpy as np

L = 192
N = L ** 3
NCORES = 8
CH = L // NCORES            # 24 planes (axis 0) per core
KH, JB = 8, 16              # partition grid: p = kh*JB + jb
JW = L // JB                # 12 j's per partition
KW = L // KH                # 24 k's per partition
IH = CH + 2                 # 26 planes incl. axis-0 halo
FJ = JW + 2                 # 14 incl. j halo strips
FK = KW + 2                 # 26 incl. k halo strips
PLF = FJ * FK               # padded plane free size (364)
FIN = IH * PLF
PF = JW * KW                # compact plane free size (288)
FOUT = CH * PF
T = 8                       # planes per compute tile
NT = CH // T
TF = T * PF

_STATE = {}


# ---------------------------------------------------------------- host side

def _shard_halo(v3pad):
    """(194,194,194) wrap-padded -> (8, 128, FIN) per-core images."""
    s0, s1, s2 = v3pad.strides
    v = np.lib.stride_tricks.as_strided(
        v3pad,
        shape=(NCORES, KH, JB, IH, FJ, FK),
        strides=(CH * s0, KW * s2, JW * s1, s0, s1, s2),
    )
    return np.ascontiguousarray(v).reshape(NCORES, 128, FIN)


def _shard_compact(v3):
    """(192,192,192) -> (8, 128, CH, PF) per-core compact images."""
    s0, s1, s2 = v3.strides
    v = np.lib.stride_tricks.as_strided(
        v3,
        shape=(NCORES, KH, JB, CH, JW, KW),
        strides=(CH * s0, KW * s2, JW * s1, s0, s1, s2),
    )
    return np.ascontiguousarray(v).reshape(NCORES, 128, CH, PF)


def _unshard_compact(per_core):
    """(8, 128, CH*PF) -> (192,192,192) float32."""
    out3 = np.empty((L, L, L), np.float32)
    s0, s1, s2 = out3.strides
    w = np.lib.stride_tricks.as_strided(
        out3,
        shape=(NCORES, KH, JB, CH, JW, KW),
        strides=(CH * s0, KW * s2, JW * s1, s0, s1, s2),
    )
    w[:] = per_core.reshape(NCORES, KH, JB, CH, JW, KW)
    return out3


def _is_const(a):
    a = np.asarray(a)
    return bool(a.size) and bool(np.all(a == a.flat[0]))


def _rolls_ok(nn_idx_1, nn_idx_2, nn_idy_1, nn_idy_2, nn_idz_1, nn_idz_2):
    """Spot-check that the index arrays are the periodic roll stencil."""
    rng = np.random.default_rng(12345)
    f = rng.integers(0, N, size=4096)
    i, r = np.divmod(f, L * L)
    j, k = np.divmod(r, L)

    def flat(ii, jj, kk):
        return (ii % L) * L * L + (jj % L) * L + (kk % L)

    checks = [
        (nn_idx_1, flat(i - 1, j, k)), (nn_idx_2, flat(i + 1, j, k)),
        (nn_idy_1, flat(i, j - 1, k)), (nn_idy_2, flat(i, j + 1, k)),
        (nn_idz_1, flat(i, j, k - 1)), (nn_idz_2, flat(i, j, k + 1)),
    ]
    for arr, want in checks:
        if not np.array_equal(np.asarray(arr)[f], want):
            return False
    return True


def _numpy_fallback(y, J, anisotropy, gamma, h_dis_x, h_dis_y, beta,
                    e_disorder, idx):
    """Exact reference math in numpy (used only if structure checks fail)."""
    x, p = y[:N], y[N:]

    def stencil(v):
        return J * (v[idx[0]] + v[idx[1]] + v[idx[2]] + v[idx[3]]
                    + anisotropy * (v[idx[4]] + v[idx[5]]))

    xL = stencil(x)
    yL = stencil(p)
    r2 = x * x + p * p
    cross = xL * p - yL * x
    dx = gamma * p * cross + e_disorder * p - yL + h_dis_y + beta * r2 * p
    dp = -gamma * x * cross - e_disorder * x + xL - h_dis_x - beta * r2 * x
    return np.concatenate([dx, dp]).astype(np.float32)


# -------------------------------------------------------------- device side

def _build_nc(anis, gJ, beta, J):
    import math

    from concourse import bacc
    import concourse.mybir as mybir
    from concourse.tile import TileContext

    ActF = mybir.ActivationFunctionType
    f16 = mybir.dt.float16
    sqb = math.sqrt(beta)

    nc = bacc.Bacc("TRN2", target_bir_lowering=False, debug=False,
                   enable_asserts=False, num_devices=NCORES)
    x_in = nc.dram_tensor("x_in", [128, FIN], f16, kind="ExternalInput").ap()
    p_in = nc.dram_tensor("p_in", [128, FIN], f16, kind="ExternalInput").ap()
    # packed per-tile coefficients: [e_disorder | h_dis_x | h_dis_y]
    cf_in = nc.dram_tensor("cf_in", [128, NT, 3, TF], f16, kind="ExternalInput").ap()
    dx_out = nc.dram_tensor("dx_out", [128, FOUT], f16, kind="ExternalOutput").ap()
    dp_out = nc.dram_tensor("dp_out", [128, FOUT], f16, kind="ExternalOutput").ap()

    with TileContext(nc) as tc:
        with (
            tc.tile_pool(name="state", bufs=2) as sp,
            tc.tile_pool(name="coef", bufs=2) as cp,
            tc.tile_pool(name="outs", bufs=2) as op_,
            tc.tile_pool(name="tmp", bufs=1) as tp,
            tc.tile_pool(name="actout", bufs=2) as ap_,
        ):
            for t in range(NT):
                f0 = t * T * PF
                i0 = t * T

                xt = sp.tile([128, (T + 2) * PLF], f16, tag="xt", name=f"xt{t}")
                if t == 0:
                    Hx = (T + 2) * PLF // 2
                    nc.sync.dma_start(xt[:, :Hx], x_in[:, :Hx])
                    nc.sync.dma_start(xt[:, Hx:], x_in[:, Hx:(T + 2) * PLF])
                else:
                    nc.sync.dma_start(xt[:], x_in[:, i0 * PLF:(i0 + T + 2) * PLF])
                pt = sp.tile([128, (T + 2) * PLF], f16, tag="pt", name=f"pt{t}")
                nc.sync.dma_start(pt[:], p_in[:, i0 * PLF:(i0 + T + 2) * PLF])
                ct = cp.tile([128, 3, TF], f16, tag="ct", name=f"ct{t}")
                nc.sync.dma_start(ct[:], cf_in[:, t])
                ed, hx, hy = ct[:, 0, :], ct[:, 1, :], ct[:, 2, :]

                def sl(img, di, dj, dk):
                    v = img[:].rearrange("q (i j k) -> q i j k",
                                         i=T + 2, j=FJ, k=FK)
                    return v[:, 1 + di: 1 + T + di,
                             1 + dj: 1 + JW + dj, 1 + dk: 1 + KW + dk]

                xc = sl(xt, 0, 0, 0)
                pc = sl(pt, 0, 0, 0)

                def v4(tile):
                    return tile[:].rearrange("q (i j k) -> q i j k",
                                             i=T, j=JW, k=KW)

                def tmp(tag):
                    return tp.tile([128, TF], f16, tag=tag, name=f"{tag}_{t}")

                def atmp(tag):
                    return ap_.tile([128, TF], f16, tag=tag, name=f"{tag}_{t}")

                add, sub, mul = (nc.vector.tensor_add, nc.vector.tensor_sub,
                                 nc.vector.tensor_mul)

                # ---- x stencil: sx = (i-sum) + (j-sum) + anis*(k-sum)
                ix, jx, kx = tmp("ix"), tmp("jx"), tmp("kx")
                kxa, sx, sxg = atmp("kxa"), tmp("sx"), atmp("sxg")
                add(v4(ix), sl(xt, -1, 0, 0), sl(xt, 1, 0, 0))
                add(v4(jx), sl(xt, 0, -1, 0), sl(xt, 0, 1, 0))
                add(v4(kx), sl(xt, 0, 0, -1), sl(xt, 0, 0, 1))
                nc.scalar.mul(kxa[:], kx[:], anis)
                add(jx[:], jx[:], kxa[:])
                add(sx[:], jx[:], ix[:])                  # sx
                nc.scalar.mul(sxg[:], sx[:], gJ)          # gJ*sx

                # ---- p stencil
                ip, jp, kp = tmp("ip"), tmp("jp"), tmp("kp")
                kpa, sp_, spg = atmp("kpa"), tmp("sp"), atmp("spg")
                add(v4(ip), sl(pt, -1, 0, 0), sl(pt, 1, 0, 0))
                add(v4(jp), sl(pt, 0, -1, 0), sl(pt, 0, 1, 0))
                add(v4(kp), sl(pt, 0, 0, -1), sl(pt, 0, 0, 1))
                nc.scalar.mul(kpa[:], kp[:], anis)
                add(jp[:], jp[:], kpa[:])
                add(sp_[:], jp[:], ip[:])                 # sp
                nc.scalar.mul(spg[:], sp_[:], gJ)         # gJ*sp

                # ---- beta*r2 via scaled squares on ACT
                sqx, sqp = atmp("sqx"), atmp("sqp")
                nc.scalar.activation(v4(sqx), xc, ActF.Square, scale=sqb)
                nc.scalar.activation(v4(sqp), pc, ActF.Square, scale=sqb)
                add(sqx[:], sqx[:], sqp[:])               # beta*r2

                # ---- s2 = gJ*(sx*p - sp*x) + e_dis + beta*r2
                w1, w2 = tmp("w1"), tmp("w2")
                mul(v4(w2), v4(spg), xc)
                mul(v4(w1), v4(sxg), pc)
                sub(w1[:], w1[:], w2[:])                  # gamma*cross
                add(w1[:], w1[:], ed)
                add(w1[:], w1[:], sqx[:])                 # s2

                # ---- dx = p*s2 + (h_y - J*sp)
                dxo = op_.tile([128, TF], f16, tag="dxo", name=f"dxo{t}")
                u1 = tmp("u1")
                if J == 1.0:
                    sub(u1[:], hy, sp_[:])
                else:
                    nc.scalar.mul(spg[:], sp_[:], J)
                    sub(u1[:], hy, spg[:])
                mul(v4(dxo), pc, v4(w1))                  # t1 = p*s2
                add(dxo[:], dxo[:], u1[:])
                nc.sync.dma_start(dx_out[:, f0:f0 + TF], dxo[:])

                # ---- dp = (J*sx - h_x) - x*s2
                dpo = op_.tile([128, TF], f16, tag="dpo", name=f"dpo{t}")
                u2 = tmp("u2")
                if J == 1.0:
                    sub(u2[:], sx[:], hx)
                else:
                    nc.scalar.mul(sxg[:], sx[:], J)
                    sub(u2[:], sxg[:], hx)
                mul(v4(dpo), xc, v4(w1))                  # t2 = x*s2
                sub(dpo[:], u2[:], dpo[:])
                nc.sync.dma_start(dp_out[:, f0:f0 + TF], dpo[:])

    nc.compile()
    return nc


def _get_nc():
    consts = _STATE.get("consts", (0.45, 0.05, 0.01, 1.0))
    key = ("nc",) + tuple(consts)
    if key not in _STATE:
        anis, gamma, beta, J = consts
        _STATE[key] = _build_nc(float(anis), float(gamma * J), float(beta),
                                float(J))
    return _STATE[key]


def _run(in_maps, trace=False, trace_cores=None):
    from concourse.bass_utils import run_bass_kernel_spmd
    if trace:
        # the agent image's antenv lacks axon_hooks; wire the NTFF hook
        import sys as _sys
        import types as _types
        if "antenv.axon_hooks" not in _sys.modules:
            try:
                import trn_agent_boot.trn_boot as _tb
                _hook = _tb._ntff_profile_via_ctypes('/opt/axon/libaxon_pjrt.so')
                _mod = _types.ModuleType("antenv.axon_hooks")
                _mod.get_axon_ntff_profile_hook = lambda: _hook
                _sys.modules["antenv.axon_hooks"] = _mod
            except Exception:
                pass
    return run_bass_kernel_spmd(
        _get_nc(), in_maps, core_ids=list(range(NCORES)),
        trace=trace, trace_cores=trace_cores,
    )


def prepare_in_maps(y, anis_v, gamma_v, beta_v, j_v, h_dis_x, h_dis_y,
                    e_disorder):
    """Host-side sharding: build the 8 per-core input maps."""
    _STATE["consts"] = (float(anis_v), float(gamma_v), float(beta_v),
                        float(j_v))
    x3 = np.ascontiguousarray(y[:N], np.float32).reshape(L, L, L)
    p3 = np.ascontiguousarray(y[N:], np.float32).reshape(L, L, L)
    xs = _shard_halo(np.pad(x3, 1, mode="wrap")).astype(np.float16)
    ps = _shard_halo(np.pad(p3, 1, mode="wrap")).astype(np.float16)
    eds = _shard_compact(np.ascontiguousarray(e_disorder, np.float32).reshape(L, L, L))
    hxs = _shard_compact(np.ascontiguousarray(h_dis_x, np.float32).reshape(L, L, L))
    hys = _shard_compact(np.ascontiguousarray(h_dis_y, np.float32).reshape(L, L, L))
    # pack per-tile coefficient blocks: [NT, 3, TF]
    cf = np.stack([eds.reshape(NCORES, 128, NT, TF),
                   hxs.reshape(NCORES, 128, NT, TF),
                   hys.reshape(NCORES, 128, NT, TF)], axis=3)
    cf = np.ascontiguousarray(cf).astype(np.float16)   # (8, 128, NT, 3, TF)
    return [
        {"x_in": xs[c], "p_in": ps[c], "cf_in": cf[c]}
        for c in range(NCORES)
    ]


def assemble_output(results):
    """Per-core device outputs -> full (2N,) float32 array."""
    dxs = np.stack([results[c]["dx_out"] for c in range(NCORES)]).astype(np.float32)
    dps = np.stack([results[c]["dp_out"] for c in range(NCORES)]).astype(np.float32)
    dx3 = _unshard_compact(dxs)
    dp3 = _unshard_compact(dps)
    return np.concatenate([dx3.reshape(-1), dp3.reshape(-1)])


def kernel(t, y, J, anisotropy, gamma, h_dis_x, h_dis_y, beta, e_disorder,
           nn_idx_1, nn_idx_2, nn_idy_1, nn_idy_2, nn_idz_1, nn_idz_2):
    y = np.asarray(y, np.float32)
    J = np.asarray(J, np.float32)
    anisotropy = np.asarray(anisotropy, np.float32)
    gamma = np.asarray(gamma, np.float32)
    beta = np.asarray(beta, np.float32)
    h_dis_x = np.asarray(h_dis_x, np.float32)
    h_dis_y = np.asarray(h_dis_y, np.float32)
    e_disorder = np.asarray(e_disorder, np.float32)

    ok = (y.shape == (2 * N,)
          and _is_const(J) and _is_const(anisotropy)
          and _is_const(gamma) and _is_const(beta)
          and _rolls_ok(nn_idx_1, nn_idx_2, nn_idy_1, nn_idy_2,
                        nn_idz_1, nn_idz_2))
    if not ok:
        idx = [np.asarray(a) for a in (nn_idx_1, nn_idx_2, nn_idy_1,
                                       nn_idy_2, nn_idz_1, nn_idz_2)]
        return _numpy_fallback(y, J, anisotropy, gamma, h_dis_x, h_dis_y,
                               beta, e_disorder, idx)

    in_maps = prepare_in_maps(
        y, float(anisotropy.flat[0]), float(gamma.flat[0]),
        float(beta.flat[0]), float(J.flat[0]), h_dis_x, h_dis_y, e_disorder)
    res = _run(in_maps, trace=False)
    return assemble_output(res.results)


# revision 12
# speedup vs baseline: 2.2592x; 1.3931x over previous
"""Bass/Trainium2 kernel for the DGPE relaxation RHS on a 192^3 periodic lattice.

The nn_id* inputs are the deterministic 6-neighbor roll indices of the
lattice, so the gathers are implemented as stencil shifts.  The lattice is
sharded along axis 0 across 8 NeuronCores (24 planes + 2 halo planes each,
sliced host-side).  Within a core, partition = (k-block, j-block) = 8 x 16,
each partition holding a (24 x 12 x 24) sub-brick stored with j/k halo
strips so every neighbor access is a plain access-pattern offset.

All device tensors are fp16: the DVE is SBUF-bandwidth bound, and 2-byte
packed operands double its throughput while also halving HBM traffic.  The
loose rel-err budget (2e-2) dwarfs fp16 rounding.  Scalar scalings (anis,
gamma*J, beta) ride the ACT engine / the ACT Square scale because the DVE
STT fused ops don't get the 2-byte fast path while plain tensor_tensor
ops do.
"""

import numpy as np

L = 192
N = L ** 3
NCORES = 8
CH = L // NCORES            # 24 planes (axis 0) per core
KH, JB = 8, 16              # partition grid: p = kh*JB + jb
JW = L // JB                # 12 j's per partition
KW = L // KH                # 24 k's per partition
IH = CH + 2                 # 26 planes incl. axis-0 halo
FJ = JW + 2                 # 14 incl. j halo strips
FK = KW + 2                 # 26 incl. k halo strips
PLF = FJ * FK               # padded plane free size (364)
FIN = IH * PLF
PF = JW * KW                # compact plane free size (288)
FOUT = CH * PF
T = 4                       # planes per compute tile
NT = CH // T
TF = T * PF

_STATE = {}


# ---------------------------------------------------------------- host side

def _shard_halo(v3pad):
    """(194,194,194) wrap-padded -> (8, 128, FIN) per-core images."""
    s0, s1, s2 = v3pad.strides
    v = np.lib.stride_tricks.as_strided(
        v3pad,
        shape=(NCORES, KH, JB, IH, FJ, FK),
        strides=(CH * s0, KW * s2, JW * s1, s0, s1, s2),
    )
    return np.ascontiguousarray(v).reshape(NCORES, 128, FIN)


def _shard_compact(v3):
    """(192,192,192) -> (8, 128, CH, PF) per-core compact images."""
    s0, s1, s2 = v3.strides
    v = np.lib.stride_tricks.as_strided(
        v3,
        shape=(NCORES, KH, JB, CH, JW, KW),
        strides=(CH * s0, KW * s2, JW * s1, s0, s1, s2),
    )
    return np.ascontiguousarray(v).reshape(NCORES, 128, CH, PF)


def _unshard_compact(per_core):
    """(8, 128, CH*PF) -> (192,192,192) float32."""
    out3 = np.empty((L, L, L), np.float32)
    s0, s1, s2 = out3.strides
    w = np.lib.stride_tricks.as_strided(
        out3,
        shape=(NCORES, KH, JB, CH, JW, KW),
        strides=(CH * s0, KW * s2, JW * s1, s0, s1, s2),
    )
    w[:] = per_core.reshape(NCORES, KH, JB, CH, JW, KW)
    return out3


def _is_const(a):
    a = np.asarray(a)
    return bool(a.size) and bool(np.all(a == a.flat[0]))


def _rolls_ok(nn_idx_1, nn_idx_2, nn_idy_1, nn_idy_2, nn_idz_1, nn_idz_2):
    """Spot-check that the index arrays are the periodic roll stencil."""
    rng = np.random.default_rng(12345)
    f = rng.integers(0, N, size=4096)
    i, r = np.divmod(f, L * L)
    j, k = np.divmod(r, L)

    def flat(ii, jj, kk):
        return (ii % L) * L * L + (jj % L) * L + (kk % L)

    checks = [
        (nn_idx_1, flat(i - 1, j, k)), (nn_idx_2, flat(i + 1, j, k)),
        (nn_idy_1, flat(i, j - 1, k)), (nn_idy_2, flat(i, j + 1, k)),
        (nn_idz_1, flat(i, j, k - 1)), (nn_idz_2, flat(i, j, k + 1)),
    ]
    for arr, want in checks:
        if not np.array_equal(np.asarray(arr)[f], want):
            return False
    return True


def _numpy_fallback(y, J, anisotropy, gamma, h_dis_x, h_dis_y, beta,
                    e_disorder, idx):
    """Exact reference math in numpy (used only if structure checks fail)."""
    x, p = y[:N], y[N:]

    def stencil(v):
        return J * (v[idx[0]] + v[idx[1]] + v[idx[2]] + v[idx[3]]
                    + anisotropy * (v[idx[4]] + v[idx[5]]))

    xL = stencil(x)
    yL = stencil(p)
    r2 = x * x + p * p
    cross = xL * p - yL * x
    dx = gamma * p * cross + e_disorder * p - yL + h_dis_y + beta * r2 * p
    dp = -gamma * x * cross - e_disorder * x + xL - h_dis_x - beta * r2 * x
    return np.concatenate([dx, dp]).astype(np.float32)


# -------------------------------------------------------------- device side

def _build_nc(anis, gJ, beta, J):
    import math
    from contextlib import ExitStack

    from concourse import bacc
    import concourse.mybir as mybir
    from concourse.bass import MemorySpace
    from concourse.masks import make_identity
    from concourse.tile import TileContext

    ActF = mybir.ActivationFunctionType
    f16 = mybir.dt.float16
    f32 = mybir.dt.float32
    sqb = math.sqrt(beta)
    gamma = gJ / J

    nc = bacc.Bacc("TRN2", target_bir_lowering=False, debug=False,
                   enable_asserts=False, num_devices=NCORES)
    x_in = nc.dram_tensor("x_in", [128, FIN], f16, kind="ExternalInput").ap()
    p_in = nc.dram_tensor("p_in", [128, FIN], f16, kind="ExternalInput").ap()
    # packed per-tile coefficients: [e_disorder | h_dis_x | h_dis_y]
    cf_in = nc.dram_tensor("cf_in", [128, NT, 3, TF], f16, kind="ExternalInput").ap()
    dx_out = nc.dram_tensor("dx_out", [128, FOUT], f16, kind="ExternalOutput").ap()
    dp_out = nc.dram_tensor("dp_out", [128, FOUT], f16, kind="ExternalOutput").ap()

    xst = ExitStack()
    with TileContext(nc) as tc:
        with (
            tc.tile_pool(name="consts", bufs=1) as kp_,
            tc.tile_pool(name="state", bufs=2) as sp,
            tc.tile_pool(name="coef", bufs=2) as cp,
            tc.tile_pool(name="outs", bufs=2) as op_,
            tc.tile_pool(name="tmp", bufs=1) as tp,
            tc.tile_pool(name="actout", bufs=2) as ap_,
            tc.tile_pool(name="psum", bufs=1, space=MemorySpace.PSUM) as pp,
        ):
            # stationary weights: J*I for in-plane shifts, J*anis*I for the
            # anisotropic k shifts (the whole 6-point stencil runs on the
            # otherwise-idle PE, accumulating in fp32 PSUM)
            wI = kp_.tile([128, 128], f16, name="wI")
            make_identity(nc, wI)
            if J != 1.0:
                wJ = kp_.tile([128, 128], f16, name="wJ")
                nc.scalar.mul(wJ[:], wI[:], J)
            else:
                wJ = wI
            wA = kp_.tile([128, 128], f16, name="wA")
            nc.scalar.mul(wA[:], wI[:], J * anis)

            for t in range(NT):
                f0 = t * T * PF
                i0 = t * T

                xt = sp.tile([128, (T + 2) * PLF], f16, tag="xt", name=f"xt{t}")
                if t == 0:
                    Hx = (T + 2) * PLF // 2
                    nc.sync.dma_start(xt[:, :Hx], x_in[:, :Hx])
                    nc.sync.dma_start(xt[:, Hx:], x_in[:, Hx:(T + 2) * PLF])
                else:
                    nc.sync.dma_start(xt[:], x_in[:, i0 * PLF:(i0 + T + 2) * PLF])
                pt = sp.tile([128, (T + 2) * PLF], f16, tag="pt", name=f"pt{t}")
                nc.sync.dma_start(pt[:], p_in[:, i0 * PLF:(i0 + T + 2) * PLF])
                ct = cp.tile([128, 3, TF], f16, tag="ct", name=f"ct{t}")
                nc.sync.dma_start(ct[:], cf_in[:, t])
                ed, hx, hy = ct[:, 0, :], ct[:, 1, :], ct[:, 2, :]

                def sl(img, di, dj, dk):
                    v = img[:].rearrange("q (i j k) -> q i j k",
                                         i=T + 2, j=FJ, k=FK)
                    return v[:, 1 + di: 1 + T + di,
                             1 + dj: 1 + JW + dj, 1 + dk: 1 + KW + dk]

                xc = sl(xt, 0, 0, 0)
                pc = sl(pt, 0, 0, 0)

                def v4(tile):
                    return tile[:].rearrange("q (i j k) -> q i j k",
                                             i=T, j=JW, k=KW)

                def tmp(tag):
                    return tp.tile([128, TF], f16, tag=tag, name=f"{tag}_{t}")

                def atmp(tag):
                    return ap_.tile([128, TF], f16, tag=tag, name=f"{tag}_{t}")

                add, sub, mul = (nc.vector.tensor_add, nc.vector.tensor_sub,
                                 nc.vector.tensor_mul)

                # ---- stencils on PE: s = J*(i-sum + j-sum + anis*k-sum),
                # per output plane 6 identity-weight matmuls accumulating
                # into one PSUM bank
                def stencil_mm(img, ps):
                    vv = img[:].rearrange("q (i j k) -> q i j k",
                                          i=T + 2, j=FJ, k=FK)
                    shifts = [(-1, 0, 0, wJ), (1, 0, 0, wJ),
                              (0, -1, 0, wJ), (0, 1, 0, wJ),
                              (0, 0, -1, wA), (0, 0, 1, wA)]
                    for i in range(T):
                        for s, (di, dj, dk, w) in enumerate(shifts):
                            rhs = vv[:, 1 + i + di,
                                     1 + dj:1 + dj + JW, 1 + dk:1 + dk + KW]
                            nc.tensor.matmul(ps[:, i, :PF], w[:], rhs,
                                             start=(s == 0), stop=(s == 5))

                psx = pp.tile([128, T, 512], f32, tag="psx", name=f"psx{t}")
                psp = pp.tile([128, T, 512], f32, tag="psp", name=f"psp{t}")
                stencil_mm(xt, psx)
                stencil_mm(pt, psp)

                # ---- ACT: scaled squares, gamma-scaled fields, PSUM
                # evacuation to packed fp16
                sqx, sqp = atmp("sqx"), atmp("sqp")
                nc.scalar.activation(v4(sqx), xc, ActF.Square, scale=sqb)
                nc.scalar.activation(v4(sqp), pc, ActF.Square, scale=sqb)
                xg, pg = atmp("xg"), atmp("pg")
                nc.scalar.mul(v4(xg), xc, gamma)
                nc.scalar.mul(v4(pg), pc, gamma)
                sxj, spj = atmp("sxj"), atmp("spj")
                nc.scalar.copy(sxj[:].rearrange("q (i f) -> q i f", i=T),
                               psx[:, :, :PF])
                nc.scalar.copy(spj[:].rearrange("q (i f) -> q i f", i=T),
                               psp[:, :, :PF])

                # ---- s2 = gJ*(sx*p - sp*x) + e_dis + beta*r2
                w1, w2 = tmp("w1"), tmp("w2")
                add(sqx[:], sqx[:], sqp[:])               # beta*r2
                mul(w2[:], spj[:], xg[:])
                mul(w1[:], sxj[:], pg[:])
                sub(w1[:], w1[:], w2[:])                  # gamma*cross
                add(w1[:], w1[:], ed)
                add(w1[:], w1[:], sqx[:])                 # s2

                # ---- dx = p*s2 + (h_y - J*sp)
                dxo = op_.tile([128, TF], f16, tag="dxo", name=f"dxo{t}")
                u1 = tmp("u1")
                sub(u1[:], hy, spj[:])
                mul(v4(dxo), pc, v4(w1))                  # t1 = p*s2
                add(dxo[:], dxo[:], u1[:])
                nc.sync.dma_start(dx_out[:, f0:f0 + TF], dxo[:])

                # ---- dp = (J*sx - h_x) - x*s2
                dpo = op_.tile([128, TF], f16, tag="dpo", name=f"dpo{t}")
                u2 = tmp("u2")
                sub(u2[:], sxj[:], hx)
                mul(v4(dpo), xc, v4(w1))                  # t2 = x*s2
                sub(dpo[:], u2[:], dpo[:])
                nc.sync.dma_start(dp_out[:, f0:f0 + TF], dpo[:])

    nc.compile()
    return nc


def _get_nc():
    consts = _STATE.get("consts", (0.45, 0.05, 0.01, 1.0))
    key = ("nc",) + tuple(consts)
    if key not in _STATE:
        anis, gamma, beta, J = consts
        _STATE[key] = _build_nc(float(anis), float(gamma * J), float(beta),
                                float(J))
    return _STATE[key]


def _run(in_maps, trace=False, trace_cores=None):
    from concourse.bass_utils import run_bass_kernel_spmd
    if trace:
        # the agent image's antenv lacks axon_hooks; wire the NTFF hook
        import sys as _sys
        import types as _types
        if "antenv.axon_hooks" not in _sys.modules:
            try:
                import trn_agent_boot.trn_boot as _tb
                _hook = _tb._ntff_profile_via_ctypes('/opt/axon/libaxon_pjrt.so')
                _mod = _types.ModuleType("antenv.axon_hooks")
                _mod.get_axon_ntff_profile_hook = lambda: _hook
                _sys.modules["antenv.axon_hooks"] = _mod
            except Exception:
                pass
    return run_bass_kernel_spmd(
        _get_nc(), in_maps, core_ids=list(range(NCORES)),
        trace=trace, trace_cores=trace_cores,
    )


def prepare_in_maps(y, anis_v, gamma_v, beta_v, j_v, h_dis_x, h_dis_y,
                    e_disorder):
    """Host-side sharding: build the 8 per-core input maps."""
    _STATE["consts"] = (float(anis_v), float(gamma_v), float(beta_v),
                        float(j_v))
    x3 = np.ascontiguousarray(y[:N], np.float32).reshape(L, L, L)
    p3 = np.ascontiguousarray(y[N:], np.float32).reshape(L, L, L)
    xs = _shard_halo(np.pad(x3, 1, mode="wrap")).astype(np.float16)
    ps = _shard_halo(np.pad(p3, 1, mode="wrap")).astype(np.float16)
    eds = _shard_compact(np.ascontiguousarray(e_disorder, np.float32).reshape(L, L, L))
    hxs = _shard_compact(np.ascontiguousarray(h_dis_x, np.float32).reshape(L, L, L))
    hys = _shard_compact(np.ascontiguousarray(h_dis_y, np.float32).reshape(L, L, L))
    # pack per-tile coefficient blocks: [NT, 3, TF]
    cf = np.stack([eds.reshape(NCORES, 128, NT, TF),
                   hxs.reshape(NCORES, 128, NT, TF),
                   hys.reshape(NCORES, 128, NT, TF)], axis=3)
    cf = np.ascontiguousarray(cf).astype(np.float16)   # (8, 128, NT, 3, TF)
    return [
        {"x_in": xs[c], "p_in": ps[c], "cf_in": cf[c]}
        for c in range(NCORES)
    ]


def assemble_output(results):
    """Per-core device outputs -> full (2N,) float32 array."""
    dxs = np.stack([results[c]["dx_out"] for c in range(NCORES)]).astype(np.float32)
    dps = np.stack([results[c]["dp_out"] for c in range(NCORES)]).astype(np.float32)
    dx3 = _unshard_compact(dxs)
    dp3 = _unshard_compact(dps)
    return np.concatenate([dx3.reshape(-1), dp3.reshape(-1)])


def kernel(t, y, J, anisotropy, gamma, h_dis_x, h_dis_y, beta, e_disorder,
           nn_idx_1, nn_idx_2, nn_idy_1, nn_idy_2, nn_idz_1, nn_idz_2):
    y = np.asarray(y, np.float32)
    J = np.asarray(J, np.float32)
    anisotropy = np.asarray(anisotropy, np.float32)
    gamma = np.asarray(gamma, np.float32)
    beta = np.asarray(beta, np.float32)
    h_dis_x = np.asarray(h_dis_x, np.float32)
    h_dis_y = np.asarray(h_dis_y, np.float32)
    e_disorder = np.asarray(e_disorder, np.float32)

    ok = (y.shape == (2 * N,)
          and _is_const(J) and _is_const(anisotropy)
          and _is_const(gamma) and _is_const(beta)
          and _rolls_ok(nn_idx_1, nn_idx_2, nn_idy_1, nn_idy_2,
                        nn_idz_1, nn_idz_2))
    if not ok:
        idx = [np.asarray(a) for a in (nn_idx_1, nn_idx_2, nn_idy_1,
                                       nn_idy_2, nn_idz_1, nn_idz_2)]
        return _numpy_fallback(y, J, anisotropy, gamma, h_dis_x, h_dis_y,
                               beta, e_disorder, idx)

    in_maps = prepare_in_maps(
        y, float(anisotropy.flat[0]), float(gamma.flat[0]),
        float(beta.flat[0]), float(J.flat[0]), h_dis_x, h_dis_y, e_disorder)
    res = _run(in_maps, trace=False)
    return assemble_output(res.results)
